# revision 82
# baseline (speedup 1.0000x reference)
"""DGCNN classification forward pass on 8 Trainium2 NeuronCores.

Strategy: data-parallel over batch B=8 (one point cloud per core); all
weights replicated.  Per core, each EdgeConv layer is reformulated as:

    h[n,k,o] = LeakyReLU(s_o * (W @ [nbr-ctr; ctr]) + b_o)
             = LeakyReLU(U[o, idx(n,k)] + V[o, n])
  with U = (s . Wn)^T p   (per point),  V = (s . (Wc-Wn))^T p + b
    out[n,o] = max_k h[n,k,o] = LeakyReLU(max_k U[o, idx(n,k)] + V[o,n])

(valid because s_o > 0 and LeakyReLU is monotone).  So each layer is:
  1) KNN keys via PE: s[i,j] = D*xxmax + 2<fi,fj> - xx_j  (row-constant
     xx_i dropped: per-row ordering is unchanged).  bf16 inputs for
     layers 2-4 (4x PE rate), with the (D*xxmax - xx_j) row folded in as
     a bf16 hi/lo rank-2 contraction so column bias error stays ~1e-4.
     All s > 0 so the f32 bit pattern is monotone in s.
  2) packed keys: one DVE scalar_tensor_tensor pass reads PSUM and
     writes key = (bits(s) & ~0x7FF) | (2047-j).  Top-32 = 4 rounds of
     max8 + 3 match_replace (no MaxIndex passes, no ACT staging copy);
     index recovered by (key & 0x7FF) ^ 0x7FF (ties prefer smaller j,
     matching jax.lax.top_k stability).
  3) neighbor gather of f16 U^T rows via SWDGE dma_gather (half the
     HBM traffic of f32)
  4) pairwise-max tree over k in f16 (2x DVE mode); V + folded bias are
     accumulated in the transpose PSUM group on PE; a single parametric-
     relu ACT op applies the leaky PSUM->feature-map.

Overlap structure (the kernel is DVE-bound on the top-32 selection):
  - keys use s' = <fi,fj> - xx_j/2 (same ordering as 2<..> - xx) so the
    moving matmul operand is the feature map itself, no scaled copy;
  - xx column sums + the -xx/2 bias row for layer l+1 are emitted per
    512-column chunk inside layer l as its output tiles complete, so a
    layer boundary only costs the kinv chain + first produce;
  - the A5 head (h = leaky(A5 @ cat + b5), [feature, point] layout) is
    dripped as [128,256] chunks between layer-4 tile consumes, its DVE
    max-reduces deferred one consume to avoid head-of-line stalls;
  - widx wrap/replicate DMAs alternate Pool/SP DGE queues; consume trees
    lag selection by 3 tiles so gathers never stall the DVE stream;
  - the global mean-pool is folded into the L1 weights host-side.
"""

import os
import sys
import numpy as np

sys.path.insert(0, "/opt/trn_rl_repo")
sys.path.insert(0, "/opt/trn_rl_repo/concourse")

import concourse.bass as bass
import concourse.bacc as bacc
import concourse.mybir as mybir
from concourse import tile
from concourse.bass_utils import run_bass_kernel_spmd

F32 = mybir.dt.float32
F32R = mybir.dt.float32r
BF16 = mybir.dt.bfloat16
U16 = mybir.dt.uint16
I16 = mybir.dt.int16
F16 = mybir.dt.float16
U32 = mybir.dt.uint32

N = 2048
K = 32
NT = N // 128  # 16 point tiles
NEG = 0.2
KEXP = 16.0  # key stretch: exp(KEXP * s / xxmax)

# layer configs: (c_in, c_out, knn_dtype)
LAYERS = [(3, 64), (64, 64), (64, 128), (128, 256)]

# debug toggles (harness never sets these; defaults = fast config)
# f32 distance matmuls cost ~400us of PE but PE is far off the critical
# path (DVE-bound kernel) and selection accuracy is 3x better than bf16.
KNN_F32 = not bool(os.environ.get("KNN_BF16"))
U_F32 = bool(os.environ.get("U_F32"))       # f32 U gather + f32 max tree
# f32r (replicated-f32) matmuls run 4x faster on PE for >=256-col outputs
# but measured bf16-grade rounding on HW: head A5 in f32r alone pushes rel
# err from 8e-3 to 2e-2.  Off by default; opt in with F32R=1 for timing
# experiments.  (CoreSim treats f32r as exact f32.)
USE_F32R = bool(os.environ.get("F32R"))
# single-op LeakyReLU on the Act engine (not implemented by CoreSim; the
# HW act tables all include leaky_relu)
USE_LRELU = not bool(os.environ.get("NO_LRELU"))

Alu = mybir.AluOpType
Act = mybir.ActivationFunctionType
Axis = mybir.AxisListType


def _leaky(nc, out, in_, accum_out=None):
    # out = max(in_ * NEG, in_) == LeakyReLU(in_, NEG)  (one DVE op)
    nc.vector.scalar_tensor_tensor(
        out, in_, NEG, in_, Alu.mult, Alu.max, accum_out=accum_out
    )


def build_module():
    nc = bacc.Bacc(
        "TRN2",
        target_bir_lowering=False,
        debug=False,
        enable_asserts=False,
        num_devices=8,
        num_swdge_queues=4,
    )

    # ---- external tensors ------------------------------------------------
    def din(name, shape):
        return nc.dram_tensor(name, list(shape), F32, kind="ExternalInput").ap()

    t_x = din("xb", (3, N))
    conv_w = []
    for li, (c, o) in enumerate(LAYERS, start=1):
        conv_w.append(
            (
                din(f"A{li}", (c, o)),
                din(f"B{li}", (c, o)),
                din(f"br{li}", (1, o)),
            )
        )
    t_A5 = [
        din("A51", (64, 1024)),
        din("A52", (64, 1024)),
        din("A53", (128, 1024)),
        din("A54a", (128, 1024)),
        din("A54b", (128, 1024)),
    ]
    t_b5 = din("b5r", (1, 1024))
    t_L1A = din("L1A", (2048, 512))
    t_b6 = din("b6r", (1, 512))
    t_L2A = din("L2A", (512, 256))
    t_L2b = din("L2br", (1, 256))
    t_L3A = din("L3A", (256, 5))
    t_L3b = din("L3br", (1, 5))
    t_F1A = din("F1A", (1024, 512))
    t_b8 = din("b8r", (1, 512))
    t_F2A = din("F2A", (512, 256))
    t_F2b = din("F2br", (1, 256))
    t_F3A = din("F3A", (256, 5))
    t_F3b = din("F3br", (1, 5))
    t_ident = din("ident", (128, 128))
    t_iotaJ = nc.dram_tensor("iotaJ", [128, N], U16, kind="ExternalInput").ap()
    t_ones512 = din("ones512", (1, 512))
    t_onesrow = din("onesrow", (1, 128))
    t_onescol = din("onescol", (128, 1))

    t_go = nc.dram_tensor("go", [5, 1], F32, kind="ExternalOutput").ap()
    t_yo = nc.dram_tensor("yo", [5, 1], F32, kind="ExternalOutput").ap()

    with tile.TileContext(nc) as tc:
        build_kernel(
            nc, tc,
            t_x, conv_w, t_A5, t_b5,
            t_L1A, t_b6, t_L2A, t_L2b, t_L3A, t_L3b,
            t_F1A, t_b8, t_F2A, t_F2b, t_F3A, t_F3b,
            t_ident, t_onesrow, t_onescol, t_go, t_yo,
            t_iotaJ, t_ones512,
        )

    nc.compile()
    return nc


def build_kernel(
    nc, tc,
    t_x, conv_w, t_A5, t_b5,
    t_L1A, t_b6, t_L2A, t_L2b, t_L3A, t_L3b,
    t_F1A, t_b8, t_F2A, t_F2b, t_F3A, t_F3b,
    t_ident, t_onesrow, t_onescol, t_go, t_yo,
    t_iotaJ, t_ones512,
):
    reps = int(os.environ.get("KERNEL_REPS", "1"))
    for _rep in range(reps):
        _build_once(
            nc, tc,
            t_x, conv_w, t_A5, t_b5,
            t_L1A, t_b6, t_L2A, t_L2b, t_L3A, t_L3b,
            t_F1A, t_b8, t_F2A, t_F2b, t_F3A, t_F3b,
            t_ident, t_onesrow, t_onescol, t_go, t_yo,
            t_iotaJ, t_ones512,
        )


def _build_once(
    nc, tc,
    t_x, conv_w, t_A5, t_b5,
    t_L1A, t_b6, t_L2A, t_L2b, t_L3A, t_L3b,
    t_F1A, t_b8, t_F2A, t_F2b, t_F3A, t_F3b,
    t_ident, t_onesrow, t_onescol, t_go, t_yo,
    t_iotaJ, t_ones512,
):
    from contextlib import ExitStack

    ctx = ExitStack()
    with ctx:
        const = ctx.enter_context(tc.tile_pool(name="const", bufs=1))
        feat = ctx.enter_context(tc.tile_pool(name="feat", bufs=1))
        dram = ctx.enter_context(tc.tile_pool(name="dram", bufs=1, space="DRAM"))

        from concourse import library_config
        nc.gpsimd.load_library(library_config.mlp)

        # constants
        ident = const.tile([128, 128], F32, tag="ident")
        nc.sync.dma_start(ident, t_ident)
        onesrow = const.tile([1, 128], F32, tag="onesrow")
        nc.sync.dma_start(onesrow, t_onesrow)
        onescol = const.tile([128, 1], F32, tag="onescol")
        nc.sync.dma_start(onescol, t_onescol)
        ones512 = const.tile([1, 512], F32, tag="ones512")
        nc.sync.dma_start(ones512, t_ones512)
        iotaJ = const.tile([128, N], U16, tag="iotaJ")
        nc.gpsimd.dma_start(iotaJ, t_iotaJ)  # off the SP queue at startup
        ones2b = const.tile([2, 128], BF16, tag="ones2b")
        nc.vector.memset(ones2b, 1.0)
        masklo16 = const.tile([128, 1], U16, tag="masklo16")
        nc.vector.memset(masklo16, 0xF800)
        masklo = const.tile([128, 1], U32, tag="masklo")
        nc.vector.memset(masklo, 0x7FF)
        inv11 = const.tile([128, K], U32, tag="inv11")
        nc.vector.memset(inv11, 0x7FF)

        # persistent feature maps (feature-major [c, N]); F0-F3 carry one
        # extra all-ones row so the f32 distance matmul can fold the
        # (-xx_j) column bias in as an extra contraction row.
        # persistent feature maps (feature-major [c, N]).  f32r matmul
        # operands must be produced pre-rounded by an engine op, so the
        # tiles are float32r-typed: F1..F4 come from the consume() ACT
        # Lrelu; F0 is ACT-rounded from a DMA staging tile.
        FDT = F32R if USE_F32R else F32
        F0 = feat.tile([3, N], FDT, tag="F0")
        if USE_F32R:
            with tc.tile_pool(name="stg0", bufs=1) as stg0:
                xstg = stg0.tile([3, N], F32, tag="xstg")
                nc.sync.dma_start(xstg, t_x)
                nc.scalar.activation(F0, xstg, Act.Copy)
        else:
            nc.sync.dma_start(F0, t_x)
        F1 = feat.tile([64, N], FDT, tag="F1")
        F2 = feat.tile([64, N], FDT, tag="F2")
        F3 = feat.tile([128, N], FDT, tag="F3")
        F4a = feat.tile([128, N], FDT, tag="F4a")
        F4b = feat.tile([128, N], FDT, tag="F4b")
        # f32r-typed all-ones rows, ACT-produced (scale=0 -> bias value;
        # input only supplies the shape and must be finite)
        onesrowR = const.tile([1, 128], FDT, tag="onesrowR")
        nc.scalar.activation(onesrowR, onesrow, Act.Copy, bias=1.0, scale=0.0)
        ones512R = const.tile([1, 512], FDT, tag="ones512R")
        nc.scalar.activation(ones512R, ones512, Act.Copy, bias=1.0, scale=0.0)

        fins = [F0, F1, F2, F3]
        fouts = [[F1], [F2], [F3], [F4a, F4b]]

        # layer weights in SBUF
        wsb = []
        for li, ((c, o), (tA, tB, tbr)) in enumerate(zip(LAYERS, conv_w), start=1):
            Asb = const.tile([c, o], F32, tag=f"A{li}", name=f"A{li}sb")
            nc.sync.dma_start(Asb, tA)
            Bsb = const.tile([c, o], F32, tag=f"B{li}", name=f"B{li}sb")
            nc.sync.dma_start(Bsb, tB)
            brsb = const.tile([1, o], F32, tag=f"br{li}", name=f"br{li}sb")
            nc.sync.dma_start(brsb, tbr)
            wsb.append((Asb, Bsb, brsb))

        # head (h = LeakyReLU(A5 @ cat + b5)) weights, loaded up front so
        # the per-column-chunk head compute can interleave with layer 4
        hw = ctx.enter_context(tc.tile_pool(name="hw", bufs=1))
        hk = ctx.enter_context(tc.tile_pool(name="hk", bufs=6))
        A5sb = []
        with tc.tile_pool(name="stgA", bufs=2) as stgA:
            for i, tA in enumerate(t_A5):
                p = 64 if i < 2 else 128
                a = hw.tile([p, 1024], FDT, tag=f"A5_{i}", name=f"A5sb{i}")
                if USE_F32R:
                    s = stgA.tile([p, 1024], F32, tag="astg", name="astg")
                    nc.sync.dma_start(s, tA)
                    nc.scalar.activation(a, s, Act.Copy)
                else:
                    # ACT DGE queue: keeps the layer-1 UTd/ld writes on SP
                    # from queueing behind 2MB of head weights at startup
                    nc.scalar.dma_start(a, tA)
                A5sb.append(a)
            b5r = hw.tile([1, 1024], FDT, tag="b5r")
            if USE_F32R:
                s = stgA.tile([1, 1024], F32, tag="bstg", name="bstg")
                nc.sync.dma_start(s, t_b5)
                nc.scalar.activation(b5r, s, Act.Copy)
            else:
                nc.sync.dma_start(b5r, t_b5)
        cmax_all = hw.tile([128, 64], F32, tag="cmax_all")
        csum_all = hw.tile([128, 64], F32, tag="csum_all")
        red_q = []

        # ------------------- EdgeConv layers -------------------
        lctx = ctx.enter_context(ExitStack())
        lw = lctx.enter_context(tc.tile_pool(name="lw", bufs=1))
        luv = lctx.enter_context(tc.tile_pool(name="luv", bufs=2))
        lk = lctx.enter_context(tc.tile_pool(name="lk", bufs=3))
        lkd = lctx.enter_context(tc.tile_pool(name="lkd", bufs=4, space="DRAM"))
        psUV = lctx.enter_context(tc.tile_pool(name="psUV", bufs=2, space="PSUM"))
        psD = lctx.enter_context(tc.tile_pool(name="psD", bufs=2, space="PSUM"))
        psT = lctx.enter_context(tc.tile_pool(name="psT", bufs=2, space="PSUM"))

        pieces = [
            (A5sb[0], F1, 64),
            (A5sb[1], F2, 64),
            (A5sb[2], F3, 128),
            (A5sb[3], F4a, 128),
            (A5sb[4], F4b, 128),
        ]

        def emit_head_unit(nch, ot):
            # one [128, 256] chunk of h = LeakyReLU(A5 @ cat + b5); PE-heavy,
            # hides under layer-4 selection (DVE) when dripped between tiles
            nsl = slice(nch * 256, (nch + 1) * 256)
            if True:
                osl = slice(ot * 128, (ot + 1) * 128)
                hps = psUV.tile([128, 256], F32, tag="uv", name="hps")
                for i, (Ax, Fx, kk) in enumerate(pieces):
                    nc.tensor.matmul(hps, Ax[:, osl], Fx[0:kk, nsl],
                                     start=(i == 0), stop=False)
                nc.tensor.matmul(hps, b5r[:, osl], ones512R[:, 0:256],
                                 start=False, stop=True)
                ci = ot * 8 + nch
                hl = hk.tile([128, 256], F16, tag="hl", name="hl")
                if USE_LRELU:
                    nc.scalar.activation(
                        hl, hps, Act.Prelu, alpha=NEG,
                        accum_out=csum_all[:, ci:ci + 1],
                    )
                else:
                    hr = hk.tile([128, 256], F32, tag="hr", name="hr")
                    nc.scalar.activation(hr, hps, Act.Relu, bias=0.0,
                                         scale=0.8)
                    nc.vector.scalar_tensor_tensor(
                        hl, hps, NEG, hr, Alu.mult, Alu.add,
                        accum_out=csum_all[:, ci:ci + 1],
                    )
                # the cmax reduce (DVE) is deferred a consume so it never
                # heads-of-line-blocks DVE on the fresh PE->ACT chain
                red_q.append((hl, ci))

        def emit_head_reduces():
            while red_q:
                hl, ci = red_q.pop(0)
                nc.vector.tensor_reduce(
                    cmax_all[:, ci:ci + 1], hl, Axis.X, Alu.max
                )

        # Chunked xx prep for the NEXT layer, emitted inside the current
        # layer as its output columns complete: PPc = Fin^2 (ACT), column
        # sums via PE, xxC chunk = -xx/2, running chunk max for kinv.
        prep_state = {}

        def emit_prep_chunk(lnext, ci):
            cn = LAYERS[lnext - 1][0]
            FinN = fins[lnext - 1]
            FinN32 = FinN.bitcast(F32) if USE_F32R else FinN
            if ci == 0:
                prep_state["xxC"] = lw.tile([1, N], F32, tag="xxc",
                                            name="xxc", bufs=2)
                prep_state["xxm4"] = lw.tile([1, 4], F32, tag="xxm4",
                                             name="xxm4", bufs=2)
            xxC = prep_state["xxC"]
            xxm4 = prep_state["xxm4"]
            cs = slice(ci * 512, (ci + 1) * 512)
            PPc = lw.tile([cn, 512], F32, tag="ppc", name="ppc", bufs=2)
            nc.scalar.activation(PPc, FinN32[0:cn, cs], Act.Square)
            xps = psUV.tile([1, 512], F32, tag="uv", name="xps")
            nc.tensor.matmul(xps, onescol[0:cn, :], PPc, start=True, stop=True)
            nc.scalar.activation(xxC[:, cs], xps, Act.Copy,
                                 bias=0.0, scale=-0.5)
            nc.vector.tensor_reduce(xxm4[:, ci:ci + 1], xps, Axis.X, Alu.max)

        for li, ((c, o), (Asb, Bsb, brsb)) in enumerate(zip(LAYERS, wsb), start=1):
            Fin = fins[li - 1]
            Fout = fouts[li - 1]
            gdt = F32 if U_F32 else F16
            # gather row must be a multiple of 256B
            PW = max(256 // mybir.dt.size(gdt), o)
            UTd = dram.tile([N, PW], gdt, tag=f"UT{li}", name=f"UT{li}d")
            kdt = F32 if (li == 1 or KNN_F32) else BF16

            # ---- per-layer KNN prep -------------------------------------
            # Keys use s' = <fi,fj> - xx_j/2 (same ordering as 2<..> - xx;
            # kinv absorbs the factor 2), so the moving operand is Fin
            # itself -- no scaled copy.  xx chunks for layers 2-4 were
            # already emitted inside the previous layer via prep hooks;
            # layer 1 emits them inline.  xx is computed from the SAME
            # values as the inner products so s'_ii - s'_ij >= 0 and every
            # point stays in its own knn set.
            Fin32 = Fin.bitcast(F32) if USE_F32R else Fin
            if kdt is F32:
                if li == 1:
                    for ci in range(4):
                        emit_prep_chunk(1, ci)
                xxC = prep_state.pop("xxC")
                xxm4 = prep_state.pop("xxm4")
                xxm1 = lw.tile([1, 1], F32, tag="xxm1", name="xxm1", bufs=2)
                nc.vector.tensor_reduce(xxm1, xxm4, Axis.X, Alu.max)
                rcp = lw.tile([1, 1], F32, tag="rcp", name="rcp", bufs=2)
                nc.vector.reciprocal(rcp, xxm1)
                kps = psUV.tile([128, 1], F32, tag="uv", name="kps")
                nc.tensor.matmul(kps, onesrow, rcp, start=True, stop=True)
                kinv = lw.tile([128, 1], F32, tag="kinv", name="kinv", bufs=2)
                nc.scalar.activation(kinv, kps, Act.Copy, bias=0.0,
                                     scale=2.0 * KEXP)
                sh1d = Fin32[0:c, :]
                sh2d = Fin32[0:c, :]
                hilo = xxC
                onesk = onesrow
            else:
                sh1 = lw.tile([c, N], BF16, tag="sh1", name="sh1")
                nc.scalar.activation(sh1, Fin32[0:c, :], Act.Copy)
                PP = lw.tile([c, N], F32, tag="pp", name="pp")
                nc.scalar.activation(PP, sh1, Act.Square)
                xxP = lw.tile([1, N], F32, tag="xxp", name="xxp")
                for ch in range(4):
                    cs = slice(ch * 512, (ch + 1) * 512)
                    xps = psUV.tile([1, 512], F32, tag="uv", name="xps")
                    nc.tensor.matmul(xps, onescol[0:c, :], PP[:, cs],
                                     start=True, stop=True)
                    nc.scalar.activation(xxP[:, cs], xps, Act.Copy)
                xxmax = lw.tile([1, 1], F32, tag="xxmax", name="xxmax")
                nc.vector.tensor_reduce(xxmax, xxP, Axis.X, Alu.max)
                rcp = lw.tile([1, 1], F32, tag="rcp", name="rcp", bufs=2)
                nc.vector.reciprocal(rcp, xxmax)
                kps = psUV.tile([128, 1], F32, tag="uv", name="kps")
                nc.tensor.matmul(kps, onesrow, rcp, start=True, stop=True)
                kinv = lw.tile([128, 1], F32, tag="kinv", name="kinv", bufs=2)
                nc.scalar.activation(kinv, kps, Act.Copy, bias=0.0, scale=KEXP)
                xxC = lw.tile([1, N], F32, tag="xxcb", name="xxcb")
                nc.scalar.activation(xxC, xxP, Act.Copy, bias=0.0, scale=-1.0)
                sh2 = lw.tile([c, N], BF16, tag="sh2b", name="sh2b")
                nc.scalar.activation(sh2, Fin32[0:c, :], Act.Copy,
                                     bias=0.0, scale=2.0)
                hilo = lw.tile([2, N], BF16, tag="hilo", name="hilo")
                nc.scalar.activation(hilo[0:1, :], xxC, Act.Copy)
                lo_t = lw.tile([1, N], BF16, tag="lo", name="lo")
                nc.vector.tensor_tensor(lo_t, xxC, hilo[0:1, :], Alu.subtract)
                nc.sync.dma_start(hilo[1:2, :], lo_t)
                onesk = ones2b
                sh1d = sh1
                sh2d = sh2

            dds_q = []
            head_q = []

            def produce(t, nc=nc, lk=lk, psD=psD, sh1d=sh1d, sh2d=sh2d,
                        hilo=hilo, onesk=onesk, iotaJ=iotaJ,
                        masklo16=masklo16, kinv=kinv, kdt=kdt):
                # keys: s = 2<fi,fj> - xx_j  (row-constant xx_i dropped),
                # stretched to exp(KEXP*s/xxmax) on ACT (monotone, >0,
                # resolution ~xxmax/(KEXP*2^12) after truncation), then
                # packed as (bits & ~0x7FF) | (2047-j) via one STT pass
                dds = lk.tile([128, N], F32, tag="dds", name="dds", bufs=4)
                for hh in range(2):
                    ddp = psD.tile([128, N // 2], F32, tag="dd", name="ddp")
                    for ch in range(2):
                        cs = slice(hh * 1024 + ch * 512,
                                   hh * 1024 + (ch + 1) * 512)
                        cp = slice(ch * 512, (ch + 1) * 512)
                        nc.tensor.matmul(
                            ddp[:, cp],
                            sh1d[:, t * 128:(t + 1) * 128],
                            sh2d[:, cs], start=True, stop=False,
                        )
                        nc.tensor.matmul(
                            ddp[:, cp], onesk,
                            hilo[:, cs], start=False, stop=True,
                        )
                    hs = slice(hh * 1024, (hh + 1) * 1024)
                    nc.scalar.activation(dds[:, hs], ddp, Act.Exp,
                                         bias=0.0, scale=kinv)
                    # pack touches only the LOW u16 of each f32 key:
                    # lo' = (lo & 0xF800) | (2047 - j), as a u16 STT on
                    # the stride-2 low-halfword lane
                    lo_v = dds.bitcast(U16)[:, 2 * hs.start:2 * hs.stop:2]
                    nc.vector.scalar_tensor_tensor(
                        lo_v, lo_v, masklo16,
                        iotaJ[:, hs], Alu.bitwise_and, Alu.bitwise_or,
                    )
                return dds

            for _pt in range(min(2, NT)):
                dds_q.append(produce(_pt))

            # U^T -> DRAM (f16, padded to PW)
            for t in range(NT):
                sl = slice(t * 128, (t + 1) * 128)
                ups = psUV.tile([128, o], F32, tag="uv", name="ups")
                nc.tensor.matmul(ups, Fin32[0:c, sl], Asb, start=True, stop=True)
                usb = luv.tile([128, PW], gdt, tag="usb", name="usb")
                nc.scalar.activation(usb[:, 0:o], ups, Act.Copy)
                if o < PW:
                    nc.scalar.activation(usb[:, o:2 * o], ups, Act.Copy)
                nc.sync.dma_start(UTd[sl, :], usb)

            # ---- KNN + gather + max per point tile ----------------------
            pend = []

            def consume(nc=nc, lk=lk, psT=psT, Fout=Fout, o=o, c=c, Fin=Fin32,
                        Bsb=Bsb, brsb=brsb, ident=ident, onesrow=onesrow):
                t, nbrv = pend.pop(0)
                sl = slice(t * 128, (t + 1) * 128)
                # max over k: pairwise-max tree; f16 levels then f32 root
                for half in (16, 8, 4, 2):
                    nc.vector.tensor_tensor(
                        nbrv[:, 0:half, 0:o],
                        nbrv[:, 0:half, 0:o],
                        nbrv[:, half:2 * half, 0:o],
                        Alu.max,
                    )
                M = lk.tile([128, o], F32, tag="m", name="mtile", bufs=2)
                nc.vector.tensor_tensor(
                    M, nbrv[:, 0, 0:o], nbrv[:, 1, 0:o], Alu.max
                )
                # transpose + V + bias in one PSUM group, leaky on ACT
                for bi, Fo in enumerate(Fout):
                    bw = min(128, o - bi * 128)
                    bsl = slice(bi * 128, bi * 128 + bw)
                    tp = psT.tile([128, 128], F32, tag="tp", name="tp")
                    nc.tensor.matmul(
                        tp[0:bw, :], M[:, bsl], ident,
                        is_transpose=True, start=True, stop=False,
                        skip_group_check=True,
                    )
                    nc.tensor.matmul(
                        tp[0:bw, :], Bsb[:, bsl], Fin[0:c, sl],
                        start=False, stop=False, skip_group_check=True,
                    )
                    nc.tensor.matmul(
                        tp[0:bw, :], brsb[:, bsl], onesrow,
                        start=False, stop=True, skip_group_check=True,
                    )
                    if USE_LRELU:
                        nc.scalar.activation(Fo[0:bw, sl], tp[0:bw, :],
                                             Act.Prelu, alpha=NEG)
                    else:
                        # LeakyReLU from PSUM: only one PSUM read allowed
                        # per DVE op, so 0.8*relu(x) on ACT then 0.2*x + r
                        rl = lk.tile([128, 128], F32, tag="rl", name="rl")
                        nc.scalar.activation(rl[0:bw, :], tp[0:bw, :],
                                             Act.Relu, bias=0.0, scale=0.8)
                        nc.vector.scalar_tensor_tensor(
                            Fo[0:bw, sl], tp[0:bw, :], NEG, rl[0:bw, :],
                            Alu.mult, Alu.add,
                        )
                return t

            for tp_i in range(0, NT, 2):
                pair = []
                for tt in range(tp_i, min(tp_i + 2, NT)):
                    dpair = dds_q.pop(0)
                    kpair = lk.tile([128, K], F32, tag="k32", name="k32")
                    pair.append((tt, dpair, kpair))
                # interleave the two tiles' dependent round chains so
                # per-op DVE drain/init overheads overlap
                for r in range(4):
                    for tt, dpx, kpx in pair:
                        nc.vector.max(kpx[:, r * 8:(r + 1) * 8], dpx)
                        if r < 3:
                            nc.vector.match_replace(
                                dpx, kpx[:, r * 8:(r + 1) * 8], dpx, 0.0
                            )
                # produce 2 tiles ahead; emitted after the rounds so the
                # pack of tile t+3 (reusing dds buf of t-1, 3-deep pool)
                # never head-of-line-blocks the current rounds on DVE
                for tt in (tp_i + 2, tp_i + 3):
                    if tt < NT:
                        dds_q.append(produce(tt))
                # per-tile tail (index extract + wrap + gather)
                for t, dds, k32 in pair:
                    sl = slice(t * 128, (t + 1) * 128)
                    # bitVec STT cannot cast, so extract to u32 and let the
                    # ld DMA read the low halfword of each u32 (LE)
                    gidx = lk.tile([128, K], U32, tag="gidx", name="gidx")
                    nc.vector.scalar_tensor_tensor(
                        gidx, k32.bitcast(U32), masklo, inv11,
                        Alu.bitwise_and, Alu.bitwise_xor,
                    )
                    # SWDGE wrapped index layout: list[i] lives at
                    # storage[i % 16, i // 16]; we need
                    # list[k*128 + p] = gidx[p, k]  =>
                    # widx[q, 8k+u] = gidx[16u+q, k]
                    ld = lkd.tile([128, K], I16, tag="ld", name="ld")
                    nc.sync.dma_start(ld, gidx.bitcast(I16)[:, 0::2])
                    widx = lk.tile([128, 256], I16, tag="widx", name="widx")
                    ldw = ld.rearrange("(u q) k -> q k u", u=8)
                    # wrap + replicate alternate between the Pool DGE queue
                    # (25ns dispatch; desc-gen waits on widx there anyway)
                    # and the lightly-loaded SP queue, so consecutive tiles'
                    # chains drain in parallel at layer tails
                    weng = nc.gpsimd if t % 2 == 0 else nc.sync
                    weng.dma_start(
                        widx[0:16, :].rearrange("q (k u) -> q k u", u=8),
                        ldw,
                    )
                    for lo, hi in ((16, 32), (32, 64), (64, 128)):
                        weng.dma_start(widx[lo:hi, :], widx[0:lo, :])

                    nbr = lk.tile([128, K * PW], gdt, tag="nbr", name="nbr",
                                  bufs=3)
                    nbrv = nbr.rearrange("p (k o) -> p k o", k=K)
                    for gc in range(4):
                        nc.gpsimd.dma_gather(
                            nbrv[:, gc * 8:(gc + 1) * 8, :],
                            UTd,
                            widx[:, gc * 64:(gc + 1) * 64],
                            1024,
                            1024,
                            PW,
                            queue_num=(t * 4 + gc) % 4,
                        )

                    pend.append((t, nbrv))
                    if len(pend) >= 3:
                        tdone = consume()
                        if li == 4:
                            emit_head_reduces()
                            if tdone % 2 == 1:
                                head_q.extend(
                                    (tdone // 2, ot) for ot in range(8))
                            for _ in range(min(5, len(head_q))):
                                emit_head_unit(*head_q.pop(0))
                        elif KNN_F32 and tdone % 4 == 3:
                            emit_prep_chunk(li + 1, tdone // 4)
            while pend:
                tdone = consume()
                if li == 4:
                    emit_head_reduces()
                    if tdone % 2 == 1:
                        head_q.extend((tdone // 2, ot) for ot in range(8))
                    for _ in range(min(5, len(head_q))):
                        emit_head_unit(*head_q.pop(0))
                elif KNN_F32 and tdone % 4 == 3:
                    emit_prep_chunk(li + 1, tdone // 4)
            if li == 4:
                while head_q:
                    emit_head_unit(*head_q.pop(0))
                emit_head_reduces()

        lctx.close()

        # ------------------- global feature + heads -------------------
        with tc.tile_pool(name="fcw", bufs=1) as fcw, \
             tc.tile_pool(name="fcwk", bufs=2) as fcwk, \
             tc.tile_pool(name="psf", bufs=4, space="PSUM") as psF:
            # split the big weight loads into j-chunks so the fc matmuls
            # (which accumulate j-sequentially) chase the DMA instead of
            # waiting for the whole 4MB/2MB tensor
            L1Asb = fcw.tile([128, 16 * 512], F32, tag="L1A", name="L1Asb")
            for jj in range(4):
                nc.sync.dma_start(
                    L1Asb.rearrange("p (j o) -> p j o", j=16)[
                        :, jj * 4:(jj + 1) * 4, :],
                    t_L1A.rearrange("(j p) o -> p j o", p=128)[
                        :, jj * 4:(jj + 1) * 4, :],
                )
            F1Asb = fcw.tile([128, 8 * 512], F32, tag="F1A", name="F1Asb")
            for jj in range(2):
                nc.scalar.dma_start(
                    F1Asb.rearrange("p (j o) -> p j o", j=8)[
                        :, jj * 4:(jj + 1) * 4, :],
                    t_F1A.rearrange("(j p) o -> p j o", p=128)[
                        :, jj * 4:(jj + 1) * 4, :],
                )
            L2Asb = fcw.tile([128, 4 * 256], F32, tag="L2A", name="L2Asb")
            nc.sync.dma_start(
                L2Asb.rearrange("p (j o) -> p j o", j=4),
                t_L2A.rearrange("(j p) o -> p j o", p=128),
            )
            F2Asb = fcw.tile([128, 4 * 256], F32, tag="F2A", name="F2Asb")
            nc.sync.dma_start(
                F2Asb.rearrange("p (j o) -> p j o", j=4),
                t_F2A.rearrange("(j p) o -> p j o", p=128),
            )
            L3Asb = fcw.tile([128, 2 * 5], F32, tag="L3A", name="L3Asb")
            nc.sync.dma_start(
                L3Asb.rearrange("p (j o) -> p j o", j=2),
                t_L3A.rearrange("(j p) o -> p j o", p=128),
            )
            F3Asb = fcw.tile([128, 2 * 5], F32, tag="F3A", name="F3Asb")
            nc.sync.dma_start(
                F3Asb.rearrange("p (j o) -> p j o", j=2),
                t_F3A.rearrange("(j p) o -> p j o", p=128),
            )
            b6sb = fcw.tile([1, 512], F32, tag="b6")
            nc.sync.dma_start(b6sb, t_b6)
            b8sb = fcw.tile([1, 512], F32, tag="b8")
            nc.sync.dma_start(b8sb, t_b8)
            L2bsb = fcw.tile([1, 256], F32, tag="L2b")
            nc.sync.dma_start(L2bsb, t_L2b)
            F2bsb = fcw.tile([1, 256], F32, tag="F2b")
            nc.sync.dma_start(F2bsb, t_F2b)
            L3bsb = fcw.tile([1, 5], F32, tag="L3b")
            nc.sync.dma_start(L3bsb, t_L3b)
            F3bsb = fcw.tile([1, 5], F32, tag="F3b")
            nc.sync.dma_start(F3bsb, t_F3b)

            # pool the per-chunk head partials computed inside layer 4
            maxh = fcw.tile([128, 8], F32, tag="maxh")
            sumh = fcw.tile([128, 8], F32, tag="sumh")
            for ot in range(8):
                nc.vector.tensor_reduce(
                    maxh[:, ot:ot + 1], cmax_all[:, ot * 8:(ot + 1) * 8],
                    Axis.X, Alu.max
                )
                nc.vector.tensor_reduce(
                    sumh[:, ot:ot + 1], csum_all[:, ot * 8:(ot + 1) * 8],
                    Axis.X, Alu.add
                )

            def fc(lhs_sb, nj, rhs_cols, bias_sb, width, out_cols, act_fn=True):
                """out[width] = (LeakyReLU?)(lhsT.T @ rhs + bias). Returns
                [128, ceil(width/128)] tile whose columns are 128-chunks."""
                nm = (width + 127) // 128
                res = fcwk.tile([128, max(nm, 1)], F32, tag=f"fc{width}_{nj}",
                                name="fcres")
                for m in range(nm):
                    mw = min(128, width - m * 128)
                    zps = psF.tile([128, 1], F32, tag="z", name="zps")
                    for j in range(nj):
                        nc.tensor.matmul(
                            zps[0:mw, :],
                            lhs_sb.rearrange("p (j o) -> p j o", j=nj)[
                                :, j, m * 128:m * 128 + mw
                            ],
                            rhs_cols[j],
                            start=(j == 0), stop=False,
                        )
                    nc.tensor.matmul(
                        zps[0:mw, :],
                        bias_sb[:, m * 128:m * 128 + mw],
                        onesrow[:, 0:1],
                        start=False, stop=True,
                    )
                    nc.scalar.activation(
                        res[0:mw, m:m + 1], zps[0:mw, :], Act.Copy
                    )
                if act_fn:
                    _leaky(nc, res, res)
                return res

            # the g (L1/L2/L3) and y (F1/F2/F3) chains are independent;
            # interleave their stages so the dependent-hop latencies of the
            # two chains overlap
            g_rhs = [maxh[:, j:j + 1] for j in range(8)] + \
                    [sumh[:, j:j + 1] for j in range(8)]
            y_rhs = [maxh[:, j:j + 1] for j in range(8)]
            z1 = fc(L1Asb, 16, g_rhs, b6sb, 512, 4)
            w1 = fc(F1Asb, 8, y_rhs, b8sb, 512, 4)
            z1_rhs = [z1[:, j:j + 1] for j in range(4)]
            w1_rhs = [w1[:, j:j + 1] for j in range(4)]
            z2 = fc(L2Asb, 4, z1_rhs, L2bsb, 256, 2)
            w2 = fc(F2Asb, 4, w1_rhs, F2bsb, 256, 2)
            z2_rhs = [z2[:, j:j + 1] for j in range(2)]
            w2_rhs = [w2[:, j:j + 1] for j in range(2)]
            z3 = fc(L3Asb, 2, z2_rhs, L3bsb, 5, 1, act_fn=False)
            w3 = fc(F3Asb, 2, w2_rhs, F3bsb, 5, 1, act_fn=False)
            nc.sync.dma_start(t_go, z3[0:5, 0:1])
            nc.sync.dma_start(t_yo, w3[0:5, 0:1])


# --------------------------------------------------------------------------
# host side
# --------------------------------------------------------------------------

_NC = None


def _get_nc():
    global _NC
    if _NC is None:
        _NC = build_module()
    return _NC


def _prep_weights(inp):
    f = lambda k: np.ascontiguousarray(np.asarray(inp[k], dtype=np.float32))
    d = {}

    for li, (c, o) in enumerate(LAYERS, start=1):
        W = f(f"W{li}")          # [o, 2c]
        s = f(f"s{li}")          # [o]
        b = f(f"b{li}")          # [o]
        Wn = W[:, :c]
        Wc = W[:, c:]
        d[f"A{li}"] = np.ascontiguousarray((s[:, None] * Wn).T)
        d[f"B{li}"] = np.ascontiguousarray((s[:, None] * (Wc - Wn)).T)
        d[f"br{li}"] = b[None, :].copy()

    A5 = np.ascontiguousarray((f("s5")[:, None] * f("W5")).T)   # [512, 1024]
    d["A51"] = A5[0:64].copy()
    d["A52"] = A5[64:128].copy()
    d["A53"] = A5[128:256].copy()
    d["A54a"] = A5[256:384].copy()
    d["A54b"] = A5[384:512].copy()
    d["b5r"] = f("b5")[None, :].copy()

    L1 = (f("s6")[:, None] * f("L1w")).T.copy()                 # [2048, 512]
    L1[1024:] /= float(N)
    d["L1A"] = np.ascontiguousarray(L1)
    d["b6r"] = f("b6")[None, :].copy()
    d["L2A"] = np.ascontiguousarray((f("s7")[:, None] * f("L2w")).T)
    d["L2br"] = (f("s7") * f("L2b") + f("b7"))[None, :].copy()
    d["L3A"] = np.ascontiguousarray(f("L3w").T)
    d["L3br"] = f("L3b")[None, :].copy()

    d["F1A"] = np.ascontiguousarray((f("s8")[:, None] * f("F1w")).T)
    d["b8r"] = f("b8")[None, :].copy()
    d["F2A"] = np.ascontiguousarray((f("s9")[:, None] * f("F2w")).T)
    d["F2br"] = (f("s9") * f("F2b") + f("b9"))[None, :].copy()
    d["F3A"] = np.ascontiguousarray(f("F3w").T)
    d["F3br"] = f("F3b")[None, :].copy()

    d["ident"] = np.eye(128, dtype=np.float32)
    # pack tie-break: low 11 bits of the key hold (2047 - j) so larger
    # key == smaller index among truncation ties (matches top_k stability)
    d["iotaJ"] = np.broadcast_to(
        (np.arange(N, dtype=np.uint16) ^ np.uint16(0x7FF))[None, :],
        (128, N)).copy()
    d["ones512"] = np.ones((1, 512), dtype=np.float32)
    d["onesrow"] = np.ones((1, 128), dtype=np.float32)
    d["onescol"] = np.ones((128, 1), dtype=np.float32)
    return d


def kernel(**inputs):
    x = np.asarray(inputs["x"], dtype=np.float32)   # [8, 3, N]
    B = x.shape[0]
    assert B == 8 and x.shape[1] == 3 and x.shape[2] == N

    shared = _prep_weights(inputs)
    in_maps = []
    for bidx in range(B):
        m = dict(shared)
        m["xb"] = np.ascontiguousarray(x[bidx])
        in_maps.append(m)

    nc = _get_nc()
    res = run_bass_kernel_spmd(nc, in_maps, core_ids=list(range(B)))
    g = np.stack([res.results[i]["go"].reshape(5) for i in range(B)])
    y = np.stack([res.results[i]["yo"].reshape(5) for i in range(B)])
    return (g.astype(np.float32), y.astype(np.float32))


if __name__ == "__main__":
    # smoke test with random data
    rng = np.random.default_rng(0)
    print("building module...")
    nc = _get_nc()
    print("built ok")



# revision 86
# speedup vs baseline: 1.1358x; 1.1358x over previous
"""DGCNN classification forward pass on 8 Trainium2 NeuronCores.

Strategy: data-parallel over batch B=8 (one point cloud per core); all
weights replicated.  Per core, each EdgeConv layer is reformulated as:

    h[n,k,o] = LeakyReLU(s_o * (W @ [nbr-ctr; ctr]) + b_o)
             = LeakyReLU(U[o, idx(n,k)] + V[o, n])
  with U = (s . Wn)^T p   (per point),  V = (s . (Wc-Wn))^T p + b
    out[n,o] = max_k h[n,k,o] = LeakyReLU(max_k U[o, idx(n,k)] + V[o,n])

(valid because s_o > 0 and LeakyReLU is monotone).  So each layer is:
  1) KNN keys via PE: s[i,j] = D*xxmax + 2<fi,fj> - xx_j  (row-constant
     xx_i dropped: per-row ordering is unchanged).  bf16 inputs for
     layers 2-4 (4x PE rate), with the (D*xxmax - xx_j) row folded in as
     a bf16 hi/lo rank-2 contraction so column bias error stays ~1e-4.
     All s > 0 so the f32 bit pattern is monotone in s.
  2) packed keys: one DVE scalar_tensor_tensor pass reads PSUM and
     writes key = (bits(s) & ~0x7FF) | (2047-j).  Top-32 = 4 rounds of
     max8 + 3 match_replace (no MaxIndex passes, no ACT staging copy);
     index recovered by (key & 0x7FF) ^ 0x7FF (ties prefer smaller j,
     matching jax.lax.top_k stability).
  3) neighbor gather of f16 U^T rows via SWDGE dma_gather (half the
     HBM traffic of f32)
  4) pairwise-max tree over k in f16 (2x DVE mode); V + folded bias are
     accumulated in the transpose PSUM group on PE; a single parametric-
     relu ACT op applies the leaky PSUM->feature-map.

Overlap structure (the kernel is DVE-bound on the top-32 selection):
  - keys use s' = <fi,fj> - xx_j/2 (same ordering as 2<..> - xx) so the
    moving matmul operand is the feature map itself, no scaled copy;
  - xx column sums + the -xx/2 bias row for layer l+1 are emitted per
    512-column chunk inside layer l as its output tiles complete, so a
    layer boundary only costs the kinv chain + first produce;
  - the A5 head (h = leaky(A5 @ cat + b5), [feature, point] layout) is
    dripped as [128,256] chunks between layer-4 tile consumes, its DVE
    max-reduces deferred one consume to avoid head-of-line stalls;
  - widx wrap/replicate DMAs alternate Pool/SP DGE queues; consume trees
    lag selection by 3 tiles so gathers never stall the DVE stream;
  - the global mean-pool is folded into the L1 weights host-side.
"""

import os
import sys
import numpy as np

sys.path.insert(0, "/opt/trn_rl_repo")
sys.path.insert(0, "/opt/trn_rl_repo/concourse")

import concourse.bass as bass
import concourse.bacc as bacc
import concourse.mybir as mybir
from concourse import tile
from concourse.bass_utils import run_bass_kernel_spmd

F32 = mybir.dt.float32
F32R = mybir.dt.float32r
BF16 = mybir.dt.bfloat16
U16 = mybir.dt.uint16
I16 = mybir.dt.int16
F16 = mybir.dt.float16
U32 = mybir.dt.uint32

N = 2048
K = 32
NT = N // 128  # 16 point tiles
NEG = 0.2
KEXP = 16.0  # key stretch: exp(KEXP * s / xxmax)

# layer configs: (c_in, c_out, knn_dtype)
LAYERS = [(3, 64), (64, 64), (64, 128), (128, 256)]

# debug toggles (harness never sets these; defaults = fast config)
# f32 distance matmuls cost ~400us of PE but PE is far off the critical
# path (DVE-bound kernel) and selection accuracy is 3x better than bf16.
KNN_F32 = not bool(os.environ.get("KNN_BF16"))
U_F32 = bool(os.environ.get("U_F32"))       # f32 U gather + f32 max tree
# f32r (replicated-f32) matmuls run 4x faster on PE for >=256-col outputs
# but measured bf16-grade rounding on HW: head A5 in f32r alone pushes rel
# err from 8e-3 to 2e-2.  Off by default; opt in with F32R=1 for timing
# experiments.  (CoreSim treats f32r as exact f32.)
USE_F32R = bool(os.environ.get("F32R"))
# single-op LeakyReLU on the Act engine (not implemented by CoreSim; the
# HW act tables all include leaky_relu)
USE_LRELU = not bool(os.environ.get("NO_LRELU"))

Alu = mybir.AluOpType
Act = mybir.ActivationFunctionType
Axis = mybir.AxisListType


def _leaky(nc, out, in_, accum_out=None):
    # out = max(in_ * NEG, in_) == LeakyReLU(in_, NEG)  (one DVE op)
    nc.vector.scalar_tensor_tensor(
        out, in_, NEG, in_, Alu.mult, Alu.max, accum_out=accum_out
    )


def build_module():
    nc = bacc.Bacc(
        "TRN2",
        target_bir_lowering=False,
        debug=False,
        enable_asserts=False,
        num_devices=8,
        num_swdge_queues=4,
    )

    # ---- external tensors ------------------------------------------------
    def din(name, shape):
        return nc.dram_tensor(name, list(shape), F32, kind="ExternalInput").ap()

    t_x = din("xb", (3, N))
    conv_w = []
    for li, (c, o) in enumerate(LAYERS, start=1):
        conv_w.append(
            (
                din(f"A{li}", (c, o)),
                din(f"B{li}", (c, o)),
                din(f"br{li}", (1, o)),
            )
        )
    t_A5 = [
        din("A51", (64, 1024)),
        din("A52", (64, 1024)),
        din("A53", (128, 1024)),
        din("A54a", (128, 1024)),
        din("A54b", (128, 1024)),
    ]
    t_b5 = din("b5r", (1, 1024))
    t_L1A = din("L1A", (2048, 512))
    t_b6 = din("b6r", (1, 512))
    t_L2A = din("L2A", (512, 256))
    t_L2b = din("L2br", (1, 256))
    t_L3A = din("L3A", (256, 5))
    t_L3b = din("L3br", (1, 5))
    t_F1A = din("F1A", (1024, 512))
    t_b8 = din("b8r", (1, 512))
    t_F2A = din("F2A", (512, 256))
    t_F2b = din("F2br", (1, 256))
    t_F3A = din("F3A", (256, 5))
    t_F3b = din("F3br", (1, 5))
    t_ident = din("ident", (128, 128))
    t_iotaJ = nc.dram_tensor("iotaJ", [128, N], U16, kind="ExternalInput").ap()
    t_ones512 = din("ones512", (1, 512))
    t_onesrow = din("onesrow", (1, 128))
    t_onescol = din("onescol", (128, 1))

    t_go = nc.dram_tensor("go", [5, 1], F32, kind="ExternalOutput").ap()
    t_yo = nc.dram_tensor("yo", [5, 1], F32, kind="ExternalOutput").ap()

    with tile.TileContext(nc) as tc:
        build_kernel(
            nc, tc,
            t_x, conv_w, t_A5, t_b5,
            t_L1A, t_b6, t_L2A, t_L2b, t_L3A, t_L3b,
            t_F1A, t_b8, t_F2A, t_F2b, t_F3A, t_F3b,
            t_ident, t_onesrow, t_onescol, t_go, t_yo,
            t_iotaJ, t_ones512,
        )

    nc.compile()
    return nc


def build_kernel(
    nc, tc,
    t_x, conv_w, t_A5, t_b5,
    t_L1A, t_b6, t_L2A, t_L2b, t_L3A, t_L3b,
    t_F1A, t_b8, t_F2A, t_F2b, t_F3A, t_F3b,
    t_ident, t_onesrow, t_onescol, t_go, t_yo,
    t_iotaJ, t_ones512,
):
    reps = int(os.environ.get("KERNEL_REPS", "1"))
    for _rep in range(reps):
        _build_once(
            nc, tc,
            t_x, conv_w, t_A5, t_b5,
            t_L1A, t_b6, t_L2A, t_L2b, t_L3A, t_L3b,
            t_F1A, t_b8, t_F2A, t_F2b, t_F3A, t_F3b,
            t_ident, t_onesrow, t_onescol, t_go, t_yo,
            t_iotaJ, t_ones512,
        )


def _build_once(
    nc, tc,
    t_x, conv_w, t_A5, t_b5,
    t_L1A, t_b6, t_L2A, t_L2b, t_L3A, t_L3b,
    t_F1A, t_b8, t_F2A, t_F2b, t_F3A, t_F3b,
    t_ident, t_onesrow, t_onescol, t_go, t_yo,
    t_iotaJ, t_ones512,
):
    from contextlib import ExitStack

    ctx = ExitStack()
    with ctx:
        const = ctx.enter_context(tc.tile_pool(name="const", bufs=1))
        feat = ctx.enter_context(tc.tile_pool(name="feat", bufs=1))
        dram = ctx.enter_context(tc.tile_pool(name="dram", bufs=1, space="DRAM"))

        from concourse import library_config
        nc.gpsimd.load_library(library_config.mlp)

        # constants; the x-input (F0) load is issued FIRST so layer-1
        # prep isn't queued behind the bulkier constants at startup
        FDT = F32R if USE_F32R else F32
        F0 = feat.tile([3, N], FDT, tag="F0")
        if not USE_F32R:
            nc.sync.dma_start(F0, t_x)
        onescol = const.tile([128, 1], F32, tag="onescol")
        nc.sync.dma_start(onescol, t_onescol)
        onesrow = const.tile([1, 128], F32, tag="onesrow")
        nc.sync.dma_start(onesrow, t_onesrow)
        ident = const.tile([128, 128], F32, tag="ident")
        nc.sync.dma_start(ident, t_ident)
        ones512 = const.tile([1, 512], F32, tag="ones512")
        nc.sync.dma_start(ones512, t_ones512)
        iotaJ = const.tile([128, N], U16, tag="iotaJ")
        nc.gpsimd.dma_start(iotaJ, t_iotaJ)  # off the SP queue at startup
        ones2b = const.tile([2, 128], BF16, tag="ones2b")
        nc.vector.memset(ones2b, 1.0)
        masklo16 = const.tile([128, 1], U16, tag="masklo16")
        nc.vector.memset(masklo16, 0xF800)
        masklo = const.tile([128, 1], U32, tag="masklo")
        nc.vector.memset(masklo, 0x7FF)
        inv11 = const.tile([128, K], U32, tag="inv11")
        nc.vector.memset(inv11, 0x7FF)

        # persistent feature maps (feature-major [c, N]).  f32r matmul
        # operands must be produced pre-rounded by an engine op, so the
        # tiles are float32r-typed: F1..F4 come from the consume() ACT
        # Lrelu; F0 is ACT-rounded from a DMA staging tile.  (F0 itself
        # was allocated + loaded at the top of the function.)
        if USE_F32R:
            with tc.tile_pool(name="stg0", bufs=1) as stg0:
                xstg = stg0.tile([3, N], F32, tag="xstg")
                nc.sync.dma_start(xstg, t_x)
                nc.scalar.activation(F0, xstg, Act.Copy)
        F1 = feat.tile([64, N], FDT, tag="F1")
        F2 = feat.tile([64, N], FDT, tag="F2")
        F3 = feat.tile([128, N], FDT, tag="F3")
        F4a = feat.tile([128, N], FDT, tag="F4a")
        F4b = feat.tile([128, N], FDT, tag="F4b")
        # f32r-typed all-ones rows, ACT-produced (scale=0 -> bias value;
        # input only supplies the shape and must be finite)
        onesrowR = const.tile([1, 128], FDT, tag="onesrowR")
        nc.scalar.activation(onesrowR, onesrow, Act.Copy, bias=1.0, scale=0.0)
        ones512R = const.tile([1, 512], FDT, tag="ones512R")
        nc.scalar.activation(ones512R, ones512, Act.Copy, bias=1.0, scale=0.0)

        fins = [F0, F1, F2, F3]
        fouts = [[F1], [F2], [F3], [F4a, F4b]]

        # layer weights in SBUF
        wsb = []
        for li, ((c, o), (tA, tB, tbr)) in enumerate(zip(LAYERS, conv_w), start=1):
            Asb = const.tile([c, o], F32, tag=f"A{li}", name=f"A{li}sb")
            nc.sync.dma_start(Asb, tA)
            Bsb = const.tile([c, o], F32, tag=f"B{li}", name=f"B{li}sb")
            nc.sync.dma_start(Bsb, tB)
            brsb = const.tile([1, o], F32, tag=f"br{li}", name=f"br{li}sb")
            nc.sync.dma_start(brsb, tbr)
            wsb.append((Asb, Bsb, brsb))

        # head (h = LeakyReLU(A5 @ cat + b5)) weights, loaded up front so
        # the per-column-chunk head compute can interleave with layer 4
        hw = ctx.enter_context(tc.tile_pool(name="hw", bufs=1))
        hk = ctx.enter_context(tc.tile_pool(name="hk", bufs=6))
        A5sb = []
        with tc.tile_pool(name="stgA", bufs=2) as stgA:
            for i, tA in enumerate(t_A5):
                p = 64 if i < 2 else 128
                a = hw.tile([p, 1024], FDT, tag=f"A5_{i}", name=f"A5sb{i}")
                if USE_F32R:
                    s = stgA.tile([p, 1024], F32, tag="astg", name="astg")
                    nc.sync.dma_start(s, tA)
                    nc.scalar.activation(a, s, Act.Copy)
                else:
                    # ACT DGE queue: keeps the layer-1 UTd/ld writes on SP
                    # from queueing behind 2MB of head weights at startup
                    nc.scalar.dma_start(a, tA)
                A5sb.append(a)
            b5r = hw.tile([1, 1024], FDT, tag="b5r")
            if USE_F32R:
                s = stgA.tile([1, 1024], F32, tag="bstg", name="bstg")
                nc.sync.dma_start(s, t_b5)
                nc.scalar.activation(b5r, s, Act.Copy)
            else:
                nc.sync.dma_start(b5r, t_b5)
        cmax_all = hw.tile([128, 64], F32, tag="cmax_all")
        csum_all = hw.tile([128, 64], F32, tag="csum_all")
        red_q = []

        # ------------------- EdgeConv layers -------------------
        lctx = ctx.enter_context(ExitStack())
        lw = lctx.enter_context(tc.tile_pool(name="lw", bufs=1))
        luv = lctx.enter_context(tc.tile_pool(name="luv", bufs=2))
        lk = lctx.enter_context(tc.tile_pool(name="lk", bufs=3))
        lkd = lctx.enter_context(tc.tile_pool(name="lkd", bufs=4, space="DRAM"))
        psUV = lctx.enter_context(tc.tile_pool(name="psUV", bufs=2, space="PSUM"))
        psD = lctx.enter_context(tc.tile_pool(name="psD", bufs=2, space="PSUM"))
        psT = lctx.enter_context(tc.tile_pool(name="psT", bufs=2, space="PSUM"))

        pieces = [
            (A5sb[0], F1, 64),
            (A5sb[1], F2, 64),
            (A5sb[2], F3, 128),
            (A5sb[3], F4a, 128),
            (A5sb[4], F4b, 128),
        ]

        def emit_head_unit(nch, ot):
            # one [128, 256] chunk of h = LeakyReLU(A5 @ cat + b5); PE-heavy,
            # hides under layer-4 selection (DVE) when dripped between tiles
            nsl = slice(nch * 256, (nch + 1) * 256)
            if True:
                osl = slice(ot * 128, (ot + 1) * 128)
                hps = psUV.tile([128, 256], F32, tag="uv", name="hps")
                for i, (Ax, Fx, kk) in enumerate(pieces):
                    nc.tensor.matmul(hps, Ax[:, osl], Fx[0:kk, nsl],
                                     start=(i == 0), stop=False)
                nc.tensor.matmul(hps, b5r[:, osl], ones512R[:, 0:256],
                                 start=False, stop=True)
                ci = ot * 8 + nch
                hl = hk.tile([128, 256], F16, tag="hl", name="hl")
                if USE_LRELU:
                    nc.scalar.activation(
                        hl, hps, Act.Prelu, alpha=NEG,
                        accum_out=csum_all[:, ci:ci + 1],
                    )
                else:
                    hr = hk.tile([128, 256], F32, tag="hr", name="hr")
                    nc.scalar.activation(hr, hps, Act.Relu, bias=0.0,
                                         scale=0.8)
                    nc.vector.scalar_tensor_tensor(
                        hl, hps, NEG, hr, Alu.mult, Alu.add,
                        accum_out=csum_all[:, ci:ci + 1],
                    )
                # the cmax reduce (DVE) is deferred a consume so it never
                # heads-of-line-blocks DVE on the fresh PE->ACT chain
                red_q.append((hl, ci))

        def emit_head_reduces():
            while red_q:
                hl, ci = red_q.pop(0)
                nc.vector.tensor_reduce(
                    cmax_all[:, ci:ci + 1], hl, Axis.X, Alu.max
                )

        # Chunked xx prep for the NEXT layer, emitted inside the current
        # layer as its output columns complete: PPc = Fin^2 (ACT), column
        # sums via PE, xxC chunk = -xx/2, running chunk max for kinv.
        prep_state = {}

        def emit_prep_chunk(lnext, ci):
            cn = LAYERS[lnext - 1][0]
            FinN = fins[lnext - 1]
            FinN32 = FinN.bitcast(F32) if USE_F32R else FinN
            if ci == 0:
                prep_state["xxC"] = lw.tile([1, N], F32, tag="xxc",
                                            name="xxc", bufs=2)
                prep_state["xxm4"] = lw.tile([1, 4], F32, tag="xxm4",
                                             name="xxm4", bufs=2)
            xxC = prep_state["xxC"]
            xxm4 = prep_state["xxm4"]
            cs = slice(ci * 512, (ci + 1) * 512)
            PPc = lw.tile([cn, 512], F32, tag="ppc", name="ppc", bufs=2)
            nc.scalar.activation(PPc, FinN32[0:cn, cs], Act.Square)
            xps = psUV.tile([1, 512], F32, tag="uv", name="xps")
            nc.tensor.matmul(xps, onescol[0:cn, :], PPc, start=True, stop=True)
            nc.scalar.activation(xxC[:, cs], xps, Act.Copy,
                                 bias=0.0, scale=-0.5)
            nc.vector.tensor_reduce(xxm4[:, ci:ci + 1], xps, Axis.X, Alu.max)

        for li, ((c, o), (Asb, Bsb, brsb)) in enumerate(zip(LAYERS, wsb), start=1):
            Fin = fins[li - 1]
            Fout = fouts[li - 1]
            gdt = F32 if U_F32 else F16
            # gather row must be a multiple of 256B
            PW = max(256 // mybir.dt.size(gdt), o)
            UTd = dram.tile([N, PW], gdt, tag=f"UT{li}", name=f"UT{li}d")
            kdt = F32 if (li == 1 or KNN_F32) else BF16

            # ---- per-layer KNN prep -------------------------------------
            # Keys use s' = <fi,fj> - xx_j/2 (same ordering as 2<..> - xx;
            # kinv absorbs the factor 2), so the moving operand is Fin
            # itself -- no scaled copy.  xx chunks for layers 2-4 were
            # already emitted inside the previous layer via prep hooks;
            # layer 1 emits them inline.  xx is computed from the SAME
            # values as the inner products so s'_ii - s'_ij >= 0 and every
            # point stays in its own knn set.
            Fin32 = Fin.bitcast(F32) if USE_F32R else Fin
            if kdt is F32:
                if li == 1:
                    for ci in range(4):
                        emit_prep_chunk(1, ci)
                xxC = prep_state.pop("xxC")
                xxm4 = prep_state.pop("xxm4")
                xxm1 = lw.tile([1, 1], F32, tag="xxm1", name="xxm1", bufs=2)
                nc.vector.tensor_reduce(xxm1, xxm4, Axis.X, Alu.max)
                rcp = lw.tile([1, 1], F32, tag="rcp", name="rcp", bufs=2)
                nc.vector.reciprocal(rcp, xxm1)
                kps = psUV.tile([128, 1], F32, tag="uv", name="kps")
                nc.tensor.matmul(kps, onesrow, rcp, start=True, stop=True)
                kinv = lw.tile([128, 1], F32, tag="kinv", name="kinv", bufs=2)
                nc.scalar.activation(kinv, kps, Act.Copy, bias=0.0,
                                     scale=2.0 * KEXP)
                sh1d = Fin32[0:c, :]
                sh2d = Fin32[0:c, :]
                hilo = xxC
                onesk = onesrow
            else:
                sh1 = lw.tile([c, N], BF16, tag="sh1", name="sh1")
                nc.scalar.activation(sh1, Fin32[0:c, :], Act.Copy)
                PP = lw.tile([c, N], F32, tag="pp", name="pp")
                nc.scalar.activation(PP, sh1, Act.Square)
                xxP = lw.tile([1, N], F32, tag="xxp", name="xxp")
                for ch in range(4):
                    cs = slice(ch * 512, (ch + 1) * 512)
                    xps = psUV.tile([1, 512], F32, tag="uv", name="xps")
                    nc.tensor.matmul(xps, onescol[0:c, :], PP[:, cs],
                                     start=True, stop=True)
                    nc.scalar.activation(xxP[:, cs], xps, Act.Copy)
                xxmax = lw.tile([1, 1], F32, tag="xxmax", name="xxmax")
                nc.vector.tensor_reduce(xxmax, xxP, Axis.X, Alu.max)
                rcp = lw.tile([1, 1], F32, tag="rcp", name="rcp", bufs=2)
                nc.vector.reciprocal(rcp, xxmax)
                kps = psUV.tile([128, 1], F32, tag="uv", name="kps")
                nc.tensor.matmul(kps, onesrow, rcp, start=True, stop=True)
                kinv = lw.tile([128, 1], F32, tag="kinv", name="kinv", bufs=2)
                nc.scalar.activation(kinv, kps, Act.Copy, bias=0.0, scale=KEXP)
                xxC = lw.tile([1, N], F32, tag="xxcb", name="xxcb")
                nc.scalar.activation(xxC, xxP, Act.Copy, bias=0.0, scale=-1.0)
                sh2 = lw.tile([c, N], BF16, tag="sh2b", name="sh2b")
                nc.scalar.activation(sh2, Fin32[0:c, :], Act.Copy,
                                     bias=0.0, scale=2.0)
                hilo = lw.tile([2, N], BF16, tag="hilo", name="hilo")
                nc.scalar.activation(hilo[0:1, :], xxC, Act.Copy)
                lo_t = lw.tile([1, N], BF16, tag="lo", name="lo")
                nc.vector.tensor_tensor(lo_t, xxC, hilo[0:1, :], Alu.subtract)
                nc.sync.dma_start(hilo[1:2, :], lo_t)
                onesk = ones2b
                sh1d = sh1
                sh2d = sh2

            dds_q = []
            head_q = []

            def produce(t, nc=nc, lk=lk, psD=psD, sh1d=sh1d, sh2d=sh2d,
                        hilo=hilo, onesk=onesk, iotaJ=iotaJ,
                        masklo16=masklo16, kinv=kinv, kdt=kdt):
                # keys: s = 2<fi,fj> - xx_j  (row-constant xx_i dropped),
                # stretched to exp(KEXP*s/xxmax) on ACT (monotone, >0,
                # resolution ~xxmax/(KEXP*2^12) after truncation), then
                # packed as (bits & ~0x7FF) | (2047-j) via one STT pass
                dds = lk.tile([128, N], F32, tag="dds", name="dds", bufs=4)
                for hh in range(2):
                    ddp = psD.tile([128, N // 2], F32, tag="dd", name="ddp")
                    for ch in range(2):
                        cs = slice(hh * 1024 + ch * 512,
                                   hh * 1024 + (ch + 1) * 512)
                        cp = slice(ch * 512, (ch + 1) * 512)
                        nc.tensor.matmul(
                            ddp[:, cp],
                            sh1d[:, t * 128:(t + 1) * 128],
                            sh2d[:, cs], start=True, stop=False,
                        )
                        nc.tensor.matmul(
                            ddp[:, cp], onesk,
                            hilo[:, cs], start=False, stop=True,
                        )
                    hs = slice(hh * 1024, (hh + 1) * 1024)
                    nc.scalar.activation(dds[:, hs], ddp, Act.Exp,
                                         bias=0.0, scale=kinv)
                    # pack touches only the LOW u16 of each f32 key:
                    # lo' = (lo & 0xF800) | (2047 - j), as a u16 STT on
                    # the stride-2 low-halfword lane
                    lo_v = dds.bitcast(U16)[:, 2 * hs.start:2 * hs.stop:2]
                    nc.vector.scalar_tensor_tensor(
                        lo_v, lo_v, masklo16,
                        iotaJ[:, hs], Alu.bitwise_and, Alu.bitwise_or,
                    )
                return dds

            for _pt in range(min(2, NT)):
                dds_q.append(produce(_pt))

            # U^T -> DRAM (f16, padded to PW)
            for t in range(NT):
                sl = slice(t * 128, (t + 1) * 128)
                ups = psUV.tile([128, o], F32, tag="uv", name="ups")
                nc.tensor.matmul(ups, Fin32[0:c, sl], Asb, start=True, stop=True)
                usb = luv.tile([128, PW], gdt, tag="usb", name="usb")
                nc.scalar.activation(usb[:, 0:o], ups, Act.Copy)
                if o < PW:
                    nc.scalar.activation(usb[:, o:2 * o], ups, Act.Copy)
                nc.sync.dma_start(UTd[sl, :], usb)

            # ---- KNN + gather + max per point tile ----------------------
            pend = []

            def consume(nc=nc, lk=lk, psT=psT, Fout=Fout, o=o, c=c, Fin=Fin32,
                        Bsb=Bsb, brsb=brsb, ident=ident, onesrow=onesrow):
                t, nbrv = pend.pop(0)
                sl = slice(t * 128, (t + 1) * 128)
                # max over k: pairwise-max tree; f16 levels then f32 root
                for half in (16, 8, 4, 2):
                    nc.vector.tensor_tensor(
                        nbrv[:, 0:half, 0:o],
                        nbrv[:, 0:half, 0:o],
                        nbrv[:, half:2 * half, 0:o],
                        Alu.max,
                    )
                M = lk.tile([128, o], F32, tag="m", name="mtile", bufs=2)
                nc.vector.tensor_tensor(
                    M, nbrv[:, 0, 0:o], nbrv[:, 1, 0:o], Alu.max
                )
                # transpose + V + bias in one PSUM group, leaky on ACT
                for bi, Fo in enumerate(Fout):
                    bw = min(128, o - bi * 128)
                    bsl = slice(bi * 128, bi * 128 + bw)
                    tp = psT.tile([128, 128], F32, tag="tp", name="tp")
                    nc.tensor.matmul(
                        tp[0:bw, :], M[:, bsl], ident,
                        is_transpose=True, start=True, stop=False,
                        skip_group_check=True,
                    )
                    nc.tensor.matmul(
                        tp[0:bw, :], Bsb[:, bsl], Fin[0:c, sl],
                        start=False, stop=False, skip_group_check=True,
                    )
                    nc.tensor.matmul(
                        tp[0:bw, :], brsb[:, bsl], onesrow,
                        start=False, stop=True, skip_group_check=True,
                    )
                    if USE_LRELU:
                        nc.scalar.activation(Fo[0:bw, sl], tp[0:bw, :],
                                             Act.Prelu, alpha=NEG)
                    else:
                        # LeakyReLU from PSUM: only one PSUM read allowed
                        # per DVE op, so 0.8*relu(x) on ACT then 0.2*x + r
                        rl = lk.tile([128, 128], F32, tag="rl", name="rl")
                        nc.scalar.activation(rl[0:bw, :], tp[0:bw, :],
                                             Act.Relu, bias=0.0, scale=0.8)
                        nc.vector.scalar_tensor_tensor(
                            Fo[0:bw, sl], tp[0:bw, :], NEG, rl[0:bw, :],
                            Alu.mult, Alu.add,
                        )
                return t

            for tp_i in range(0, NT, 2):
                pair = []
                for tt in range(tp_i, min(tp_i + 2, NT)):
                    dpair = dds_q.pop(0)
                    kpair = lk.tile([128, K], F32, tag="k32", name="k32")
                    pair.append((tt, dpair, kpair))
                # interleave the two tiles' dependent round chains so
                # per-op DVE drain/init overheads overlap
                for r in range(4):
                    for tt, dpx, kpx in pair:
                        nc.vector.max(kpx[:, r * 8:(r + 1) * 8], dpx)
                        if r < 3:
                            nc.vector.match_replace(
                                dpx, kpx[:, r * 8:(r + 1) * 8], dpx, 0.0
                            )
                # produce 2 tiles ahead; emitted after the rounds so the
                # pack of tile t+3 (reusing dds buf of t-1, 3-deep pool)
                # never head-of-line-blocks the current rounds on DVE
                for tt in (tp_i + 2, tp_i + 3):
                    if tt < NT:
                        dds_q.append(produce(tt))
                # per-tile tail (index extract + wrap + gather)
                for t, dds, k32 in pair:
                    sl = slice(t * 128, (t + 1) * 128)
                    # bitVec STT cannot cast, so extract to u32 and let the
                    # ld DMA read the low halfword of each u32 (LE)
                    gidx = lk.tile([128, K], U32, tag="gidx", name="gidx")
                    nc.vector.scalar_tensor_tensor(
                        gidx, k32.bitcast(U32), masklo, inv11,
                        Alu.bitwise_and, Alu.bitwise_xor,
                    )
                    # SWDGE wrapped index layout: list[i] lives at
                    # storage[i % 16, i // 16]; we need
                    # list[k*128 + p] = gidx[p, k]  =>
                    # widx[q, 8k+u] = gidx[16u+q, k]
                    ld = lkd.tile([128, K], I16, tag="ld", name="ld")
                    nc.sync.dma_start(ld, gidx.bitcast(I16)[:, 0::2])
                    widx = lk.tile([128, 256], I16, tag="widx", name="widx")
                    ldw = ld.rearrange("(u q) k -> q k u", u=8)
                    # wrap + replicate alternate between the Pool DGE queue
                    # (25ns dispatch; desc-gen waits on widx there anyway)
                    # and the lightly-loaded SP queue, so consecutive tiles'
                    # chains drain in parallel at layer tails
                    weng = nc.gpsimd if t % 2 == 0 else nc.sync
                    weng.dma_start(
                        widx[0:16, :].rearrange("q (k u) -> q k u", u=8),
                        ldw,
                    )
                    for lo, hi in ((16, 32), (32, 64), (64, 128)):
                        weng.dma_start(widx[lo:hi, :], widx[0:lo, :])

                    nbr = lk.tile([128, K * PW], gdt, tag="nbr", name="nbr",
                                  bufs=3)
                    nbrv = nbr.rearrange("p (k o) -> p k o", k=K)
                    for gc in range(4):
                        nc.gpsimd.dma_gather(
                            nbrv[:, gc * 8:(gc + 1) * 8, :],
                            UTd,
                            widx[:, gc * 64:(gc + 1) * 64],
                            1024,
                            1024,
                            PW,
                            queue_num=(t * 4 + gc) % 4,
                        )

                    pend.append((t, nbrv))
                    if len(pend) >= 3:
                        tdone = consume()
                        if li == 4:
                            emit_head_reduces()
                            if tdone % 2 == 1:
                                head_q.extend(
                                    (tdone // 2, ot) for ot in range(8))
                            for _ in range(min(5, len(head_q))):
                                emit_head_unit(*head_q.pop(0))
                        elif KNN_F32 and tdone % 4 == 3:
                            emit_prep_chunk(li + 1, tdone // 4)
            while pend:
                tdone = consume()
                if li == 4:
                    emit_head_reduces()
                    if tdone % 2 == 1:
                        head_q.extend((tdone // 2, ot) for ot in range(8))
                    for _ in range(min(5, len(head_q))):
                        emit_head_unit(*head_q.pop(0))
                elif KNN_F32 and tdone % 4 == 3:
                    emit_prep_chunk(li + 1, tdone // 4)
            if li == 4:
                while head_q:
                    emit_head_unit(*head_q.pop(0))
                emit_head_reduces()

        lctx.close()

        # ------------------- global feature + heads -------------------
        with tc.tile_pool(name="fcw", bufs=1) as fcw, \
             tc.tile_pool(name="fcwk", bufs=2) as fcwk, \
             tc.tile_pool(name="psf", bufs=4, space="PSUM") as psF:
            # split the big weight loads into j-chunks so the fc matmuls
            # (which accumulate j-sequentially) chase the DMA instead of
            # waiting for the whole 4MB/2MB tensor
            L1Asb = fcw.tile([128, 16 * 512], F32, tag="L1A", name="L1Asb")
            for jj in range(4):
                nc.sync.dma_start(
                    L1Asb.rearrange("p (j o) -> p j o", j=16)[
                        :, jj * 4:(jj + 1) * 4, :],
                    t_L1A.rearrange("(j p) o -> p j o", p=128)[
                        :, jj * 4:(jj + 1) * 4, :],
                )
            F1Asb = fcw.tile([128, 8 * 512], F32, tag="F1A", name="F1Asb")
            for jj in range(2):
                nc.scalar.dma_start(
                    F1Asb.rearrange("p (j o) -> p j o", j=8)[
                        :, jj * 4:(jj + 1) * 4, :],
                    t_F1A.rearrange("(j p) o -> p j o", p=128)[
                        :, jj * 4:(jj + 1) * 4, :],
                )
            L2Asb = fcw.tile([128, 4 * 256], F32, tag="L2A", name="L2Asb")
            nc.sync.dma_start(
                L2Asb.rearrange("p (j o) -> p j o", j=4),
                t_L2A.rearrange("(j p) o -> p j o", p=128),
            )
            F2Asb = fcw.tile([128, 4 * 256], F32, tag="F2A", name="F2Asb")
            nc.sync.dma_start(
                F2Asb.rearrange("p (j o) -> p j o", j=4),
                t_F2A.rearrange("(j p) o -> p j o", p=128),
            )
            L3Asb = fcw.tile([128, 2 * 5], F32, tag="L3A", name="L3Asb")
            nc.sync.dma_start(
                L3Asb.rearrange("p (j o) -> p j o", j=2),
                t_L3A.rearrange("(j p) o -> p j o", p=128),
            )
            F3Asb = fcw.tile([128, 2 * 5], F32, tag="F3A", name="F3Asb")
            nc.sync.dma_start(
                F3Asb.rearrange("p (j o) -> p j o", j=2),
                t_F3A.rearrange("(j p) o -> p j o", p=128),
            )
            b6sb = fcw.tile([1, 512], F32, tag="b6")
            nc.sync.dma_start(b6sb, t_b6)
            b8sb = fcw.tile([1, 512], F32, tag="b8")
            nc.sync.dma_start(b8sb, t_b8)
            L2bsb = fcw.tile([1, 256], F32, tag="L2b")
            nc.sync.dma_start(L2bsb, t_L2b)
            F2bsb = fcw.tile([1, 256], F32, tag="F2b")
            nc.sync.dma_start(F2bsb, t_F2b)
            L3bsb = fcw.tile([1, 5], F32, tag="L3b")
            nc.sync.dma_start(L3bsb, t_L3b)
            F3bsb = fcw.tile([1, 5], F32, tag="F3b")
            nc.sync.dma_start(F3bsb, t_F3b)

            # pool the per-chunk head partials computed inside layer 4
            maxh = fcw.tile([128, 8], F32, tag="maxh")
            sumh = fcw.tile([128, 8], F32, tag="sumh")
            for ot in range(8):
                nc.vector.tensor_reduce(
                    maxh[:, ot:ot + 1], cmax_all[:, ot * 8:(ot + 1) * 8],
                    Axis.X, Alu.max
                )
                nc.vector.tensor_reduce(
                    sumh[:, ot:ot + 1], csum_all[:, ot * 8:(ot + 1) * 8],
                    Axis.X, Alu.add
                )

            def fc(lhs_sb, nj, rhs_cols, bias_sb, width, out_cols, act_fn=True):
                """out[width] = (LeakyReLU?)(lhsT.T @ rhs + bias). Returns
                [128, ceil(width/128)] tile whose columns are 128-chunks."""
                nm = (width + 127) // 128
                res = fcwk.tile([128, max(nm, 1)], F32, tag=f"fc{width}_{nj}",
                                name="fcres")
                for m in range(nm):
                    mw = min(128, width - m * 128)
                    zps = psF.tile([128, 1], F32, tag="z", name="zps")
                    for j in range(nj):
                        nc.tensor.matmul(
                            zps[0:mw, :],
                            lhs_sb.rearrange("p (j o) -> p j o", j=nj)[
                                :, j, m * 128:m * 128 + mw
                            ],
                            rhs_cols[j],
                            start=(j == 0), stop=False,
                        )
                    nc.tensor.matmul(
                        zps[0:mw, :],
                        bias_sb[:, m * 128:m * 128 + mw],
                        onesrow[:, 0:1],
                        start=False, stop=True,
                    )
                    nc.scalar.activation(
                        res[0:mw, m:m + 1], zps[0:mw, :], Act.Copy
                    )
                if act_fn:
                    _leaky(nc, res, res)
                return res

            # the g (L1/L2/L3) and y (F1/F2/F3) chains are independent;
            # interleave their stages so the dependent-hop latencies of the
            # two chains overlap
            g_rhs = [maxh[:, j:j + 1] for j in range(8)] + \
                    [sumh[:, j:j + 1] for j in range(8)]
            y_rhs = [maxh[:, j:j + 1] for j in range(8)]
            z1 = fc(L1Asb, 16, g_rhs, b6sb, 512, 4)
            w1 = fc(F1Asb, 8, y_rhs, b8sb, 512, 4)
            z1_rhs = [z1[:, j:j + 1] for j in range(4)]
            w1_rhs = [w1[:, j:j + 1] for j in range(4)]
            z2 = fc(L2Asb, 4, z1_rhs, L2bsb, 256, 2)
            w2 = fc(F2Asb, 4, w1_rhs, F2bsb, 256, 2)
            z2_rhs = [z2[:, j:j + 1] for j in range(2)]
            w2_rhs = [w2[:, j:j + 1] for j in range(2)]
            z3 = fc(L3Asb, 2, z2_rhs, L3bsb, 5, 1, act_fn=False)
            w3 = fc(F3Asb, 2, w2_rhs, F3bsb, 5, 1, act_fn=False)
            nc.sync.dma_start(t_go, z3[0:5, 0:1])
            nc.sync.dma_start(t_yo, w3[0:5, 0:1])


# --------------------------------------------------------------------------
# host side
# --------------------------------------------------------------------------

_NC = None


def _get_nc():
    global _NC
    if _NC is None:
        _NC = build_module()
    return _NC


def _prep_weights(inp):
    f = lambda k: np.ascontiguousarray(np.asarray(inp[k], dtype=np.float32))
    d = {}

    for li, (c, o) in enumerate(LAYERS, start=1):
        W = f(f"W{li}")          # [o, 2c]
        s = f(f"s{li}")          # [o]
        b = f(f"b{li}")          # [o]
        Wn = W[:, :c]
        Wc = W[:, c:]
        d[f"A{li}"] = np.ascontiguousarray((s[:, None] * Wn).T)
        d[f"B{li}"] = np.ascontiguousarray((s[:, None] * (Wc - Wn)).T)
        d[f"br{li}"] = b[None, :].copy()

    A5 = np.ascontiguousarray((f("s5")[:, None] * f("W5")).T)   # [512, 1024]
    d["A51"] = A5[0:64].copy()
    d["A52"] = A5[64:128].copy()
    d["A53"] = A5[128:256].copy()
    d["A54a"] = A5[256:384].copy()
    d["A54b"] = A5[384:512].copy()
    d["b5r"] = f("b5")[None, :].copy()

    L1 = (f("s6")[:, None] * f("L1w")).T.copy()                 # [2048, 512]
    L1[1024:] /= float(N)
    d["L1A"] = np.ascontiguousarray(L1)
    d["b6r"] = f("b6")[None, :].copy()
    d["L2A"] = np.ascontiguousarray((f("s7")[:, None] * f("L2w")).T)
    d["L2br"] = (f("s7") * f("L2b") + f("b7"))[None, :].copy()
    d["L3A"] = np.ascontiguousarray(f("L3w").T)
    d["L3br"] = f("L3b")[None, :].copy()

    d["F1A"] = np.ascontiguousarray((f("s8")[:, None] * f("F1w")).T)
    d["b8r"] = f("b8")[None, :].copy()
    d["F2A"] = np.ascontiguousarray((f("s9")[:, None] * f("F2w")).T)
    d["F2br"] = (f("s9") * f("F2b") + f("b9"))[None, :].copy()
    d["F3A"] = np.ascontiguousarray(f("F3w").T)
    d["F3br"] = f("F3b")[None, :].copy()

    d["ident"] = np.eye(128, dtype=np.float32)
    # pack tie-break: low 11 bits of the key hold (2047 - j) so larger
    # key == smaller index among truncation ties (matches top_k stability)
    d["iotaJ"] = np.broadcast_to(
        (np.arange(N, dtype=np.uint16) ^ np.uint16(0x7FF))[None, :],
        (128, N)).copy()
    d["ones512"] = np.ones((1, 512), dtype=np.float32)
    d["onesrow"] = np.ones((1, 128), dtype=np.float32)
    d["onescol"] = np.ones((128, 1), dtype=np.float32)
    return d


def kernel(**inputs):
    x = np.asarray(inputs["x"], dtype=np.float32)   # [8, 3, N]
    B = x.shape[0]
    assert B == 8 and x.shape[1] == 3 and x.shape[2] == N

    shared = _prep_weights(inputs)
    in_maps = []
    for bidx in range(B):
        m = dict(shared)
        m["xb"] = np.ascontiguousarray(x[bidx])
        in_maps.append(m)

    nc = _get_nc()
    res = run_bass_kernel_spmd(nc, in_maps, core_ids=list(range(B)))
    g = np.stack([res.results[i]["go"].reshape(5) for i in range(B)])
    y = np.stack([res.results[i]["yo"].reshape(5) for i in range(B)])
    return (g.astype(np.float32), y.astype(np.float32))


if __name__ == "__main__":
    # smoke test with random data
    rng = np.random.default_rng(0)
    print("building module...")
    nc = _get_nc()
    print("built ok")

